# revision 8
# baseline (speedup 1.0000x reference)
"""Trainium2 Bass kernel for the NeuralSDE Q-model scan.

Strategy
--------
Data-parallel over paths: 65536 paths -> 8 cores x 8192 paths. No
cross-core communication.

The per-step MLP+tanh (lambda = 3*tanh(MLP(log_v, t))) is replaced by a
per-step degree-D polynomial in xi = (log_v + 2.5)/4.5, fit on host from
the (input-provided) MLP weights over the full clipped range
log_v in [-7, 2]. Max |error| of the fit at D=13 is ~2e-5 in tanh units,
far inside fp32 tolerance for all three outputs.

Device-side, per core (8192 paths as (128 partitions x 64 free)):

Phase A (sequential over 512 steps, DVE-dominated):
    xi  = (x + 2.5)/4.5                     [tensor_scalar]
    r   = Horner chain in xi                [tensor_scalar + scalar_tensor_tensor]
    v   = alpha*x + r                       [scalar_tensor_tensor]
    w   = s1*z1_t + v                       [scalar_tensor_tensor]
    x'  = clip(w + (beta+gamma0), -7, 2)    [2x tensor_scalar]
    Q  += (s*(r+gamma0))^2                  [ACT Square + scalar_tensor_tensor]
    x' is written into an SBUF-resident history stack (128, 64, 513).

Phase B (bulk, per path-block b of 128 paths, steps contiguous):
    vol = |c2|*exp(x/2)                     [ACT Exp]
    sqv = (dt/2)*exp(x)                     [ACT Square]
    zs  = z1 + (c3/c2)*z2                   [scalar_tensor_tensor]
    m   = -vol*zs                           [scalar_tensor_tensor]
    d0  = R*dt - sqv                        [tensor_scalar]
    ls  = cumsum_t(d0 + m)                  [tensor_tensor_scan]
    spot= exp(ls)                           [ACT Exp]
    DMA out log_v rows and spot rows (path-major, contiguous).

z1 is additionally passed in step-major layout (host transpose) so the
sequential phase reads contiguous 32KB rows.
"""

import os
import sys

if "/opt/trn_rl_repo" not in sys.path:
    sys.path.insert(0, "/opt/trn_rl_repo")

import numpy as np

import concourse.bass as bass
import concourse.mybir as mybir
from concourse import tile
from concourse.bass_utils import run_bass_kernel_spmd

Alu = mybir.AluOpType
Act = mybir.ActivationFunctionType
F32 = mybir.dt.float32

# ---- problem constants (hardcoded; must match the reference) ----
N_PATHS = 65536
N_STEPS = 512
DT = 1.0 / 252.0
KAPPA = 2.72
THETA = -3.5
SIGMA_P = (0.1 + 1.6) / 2.0
RHO = -0.85
R = 0.0373
LAMBDA_MAX = 3.0
LOG_V_MIN = -7.0
LOG_V_MAX = 2.0

N_CORES = 8
P_LOCAL = N_PATHS // N_CORES          # 8192
NQ = 128                              # partitions
NF = P_LOCAL // NQ                    # 64 path-blocks (free dim)

DEGREE = int(os.environ.get("NSDE_DEGREE", "13"))
PHASES = os.environ.get("NSDE_PHASES", "ab")   # debug knob: "a" skips phase B
QACC_ON = os.environ.get("NSDE_QACC", "1") == "1"
XI_M = -2.5                           # xi = (x - XI_M)/XI_H
XI_H = 4.5

ALPHA = 1.0 - KAPPA * DT
BETA = KAPPA * THETA * DT
S1 = SIGMA_P * np.sqrt(DT)            # noise scale on log_v
CA = LAMBDA_MAX * SIGMA_P * DT        # p~ = -CA * tanh(raw)
SQ_SCALE = LAMBDA_MAX * np.sqrt(DT) / CA   # == 1/(sigma*sqrt(dt)); (SQ_SCALE*p~)^2 == lambda^2*dt
C2 = RHO * np.sqrt(DT)                # < 0
C3 = np.sqrt(1.0 - RHO * RHO) * np.sqrt(DT)
C32 = C3 / C2
VOL_BIAS = float(np.log(abs(C2)))     # vol_s = |c2| * exp(x/2)
SQV_SCALE = float(np.sqrt(DT / 2.0) / abs(C2))  # Square(SQV_SCALE*vol_s) = dt/2*e^x
RDT = R * DT

MAX_WAITS = 1


def _gelu_tanh(x):
    return 0.5 * x * (1.0 + np.tanh(np.sqrt(2 / np.pi) * (x + 0.044715 * x ** 3)))


def _fit_polys(W1, b1, W2, b2, W3, b3, n_steps, degree):
    """Per-step polynomial coefficients (power basis in xi), float64.

    Returns gam: (n_steps, degree+1); p~_t(xi) = sum_k gam[t,k] xi^k
    approximates -CA * tanh(raw_mlp(x, t_n)).
    """
    W1 = W1.astype(np.float64); b1 = b1.astype(np.float64)
    W2 = W2.astype(np.float64); b2 = b2.astype(np.float64)
    W3 = W3.astype(np.float64); b3 = b3.astype(np.float64)
    t_idx = np.arange(n_steps, dtype=np.float64) * DT / (n_steps * DT)

    # Chebyshev-node sampling on [-7, 2]
    G = max(4 * (degree + 1), 96)
    k = np.arange(G)
    xg = np.cos(np.pi * (k + 0.5) / G)           # [-1, 1]
    x = XI_M + XI_H * xg                          # [-7, 2]

    # raw(x, t) for all t at once: (T, G)
    X = np.stack([np.repeat(x[None, :], n_steps, 0),
                  np.repeat(t_idx[:, None], G, 1)], axis=-1)  # (T, G, 2)
    h = _gelu_tanh(X @ W1.T + b1)
    h = _gelu_tanh(h @ W2.T + b2)
    raw = (h @ W3.T + b3)[..., 0]                 # (T, G)
    f = -CA * np.tanh(raw)                        # target, scaled

    # Chebyshev fit in xg per t, then convert to power basis in xi (==xg).
    V = np.polynomial.chebyshev.chebvander(xg, degree)      # (G, D+1)
    coef, *_ = np.linalg.lstsq(V, f.T, rcond=None)          # (D+1, T)
    gam = np.empty((n_steps, degree + 1), np.float64)
    for t in range(n_steps):
        gam[t] = np.polynomial.chebyshev.cheb2poly(coef[:, t])
    return gam


def _split_waits(nc):
    """Workaround: this walrus build allows only one sync-wait per
    instruction; push extra waits onto preceding same-engine NoOps."""
    for bb in nc.main_func.blocks:
        newlist = []
        for ins in bb.instructions:
            si = ins.sync_info
            if si is not None and si.on_wait and len(si.on_wait) > MAX_WAITS:
                waits = list(si.on_wait)
                extra, keep = waits[:-MAX_WAITS], waits[-MAX_WAITS:]
                for i, w in enumerate(extra):
                    nop = mybir.InstNoOp(
                        name=f"{ins.name}-ws{i}", engine=ins.engine,
                        ins=[], outs=[],
                        sync_info=mybir.SyncInfo(on_wait=[w], on_update=[]))
                    nc.register_instruction(nop, overwrite=True)
                    newlist.append(nop)
                ins.sync_info = mybir.SyncInfo(on_wait=keep,
                                               on_update=list(si.on_update))
            newlist.append(ins)
        bb.instructions[:] = newlist


def build_program(gam, init_log_v, n_steps):
    """Emit the bass program. gam: (n_steps, D+1) float64."""
    D = gam.shape[1] - 1
    nc = bass.Bass()

    z1m = nc.dram_tensor("z1m", [P_LOCAL, N_STEPS], F32, kind="ExternalInput")
    z2m = nc.dram_tensor("z2m", [P_LOCAL, N_STEPS], F32, kind="ExternalInput")
    z1t = nc.dram_tensor("z1t", [N_STEPS, P_LOCAL], F32, kind="ExternalInput")
    sqbias = nc.dram_tensor("sqbias", [NQ, N_STEPS], F32, kind="ExternalInput")
    lv_out = nc.dram_tensor("lv", [P_LOCAL, N_STEPS], F32, kind="ExternalOutput")
    sp_out = nc.dram_tensor("sp", [P_LOCAL, N_STEPS], F32, kind="ExternalOutput")
    q_out = nc.dram_tensor("lsq", [P_LOCAL], F32, kind="ExternalOutput")

    # DRAM views
    z1m_v = z1m.rearrange("(q f) s -> q f s", q=NQ)   # [128, 64, 512]
    z2m_v = z2m.rearrange("(q f) s -> q f s", q=NQ)
    lv_v = lv_out.rearrange("(q f) s -> q f s", q=NQ)
    sp_v = sp_out.rearrange("(q f) s -> q f s", q=NQ)
    z1t_v = z1t.rearrange("s (q f) -> s q f", q=NQ)   # [512, 128, 64]
    q_v = q_out.rearrange("(q f) -> q f", q=NQ)       # [128, 64]

    with tile.TileContext(nc) as tc:
        with (
            tc.tile_pool(name="stack", bufs=1) as stack_pool,
            tc.tile_pool(name="state", bufs=1) as state_pool,
            tc.tile_pool(name="work", bufs=2) as work,
            tc.tile_pool(name="zin", bufs=4) as zin,
            tc.tile_pool(name="bulk", bufs=3) as bulk,
            tc.tile_pool(name="bout", bufs=3) as bout,
            tc.tile_pool(name="consts", bufs=1) as consts,
        ):
            stack = stack_pool.tile([NQ, NF, n_steps + 1], F32, tag="stack")
            qacc = state_pool.tile([NQ, NF], F32, tag="qacc")
            sqb = consts.tile([NQ, N_STEPS], F32, tag="sqb")
            zerocol = consts.tile([NQ, 1], F32, tag="zerocol")
            volb = consts.tile([NQ, 1], F32, tag="volb")

            nc.sync.dma_start(sqb[:], sqbias[:])
            nc.vector.memset(zerocol[:], 0.0)
            nc.vector.memset(volb[:], VOL_BIAS)
            nc.vector.memset(qacc[:], 0.0)
            nc.vector.memset(stack[:, :, 0], float(init_log_v))

            # ---------------- Phase A: sequential scan ----------------
            for t in range(n_steps):
                g = gam[t]
                xt = stack[:, :, t]
                z1tile = zin.tile([NQ, NF], F32, tag="z1tile")
                nc.sync.dma_start(z1tile[:], z1t_v[t])

                xi = work.tile([NQ, NF], F32, tag="xi")
                nc.vector.tensor_scalar(
                    xi[:], xt, 1.0 / XI_H, -XI_M / XI_H, Alu.mult, Alu.add)
                r = work.tile([NQ, NF], F32, tag="r")
                nc.vector.tensor_scalar(
                    r[:], xi[:], float(g[D]), float(g[D - 1]), Alu.mult, Alu.add)
                # shifted Horner: r = (r + a) * xi ; a runs g[D-2] .. g[1]
                for k in range(D - 2, 0, -1):
                    nc.vector.scalar_tensor_tensor(
                        r[:], r[:], float(g[k]), xi[:], Alu.add, Alu.mult)
                # v = alpha*x + r
                v = work.tile([NQ, NF], F32, tag="v")
                nc.vector.scalar_tensor_tensor(
                    v[:], xt, ALPHA, r[:], Alu.mult, Alu.add)
                # w = s1*z1 + v
                w = work.tile([NQ, NF], F32, tag="w")
                nc.vector.scalar_tensor_tensor(
                    w[:], z1tile[:], float(S1), v[:], Alu.mult, Alu.add)
                # x' = clip(w + (beta + g0))
                x1 = work.tile([NQ, NF], F32, tag="x1")
                nc.vector.tensor_scalar(
                    x1[:], w[:], float(BETA + g[0]), LOG_V_MIN, Alu.add, Alu.max)
                nc.vector.tensor_scalar_min(stack[:, :, t + 1], x1[:], LOG_V_MAX)
                if QACC_ON:
                    # Q += (s*(r + g0))^2  via ACT Square then DVE accumulate
                    sq = work.tile([NQ, NF], F32, tag="sq")
                    nc.scalar.activation(
                        sq[:], r[:], Act.Square,
                        bias=sqb[:, t:t + 1], scale=float(SQ_SCALE))
                    nc.vector.scalar_tensor_tensor(
                        qacc[:], sq[:], 1.0, qacc[:], Alu.mult, Alu.add)

            nc.sync.dma_start(q_v[:, :], qacc[:])

            # ---------------- Phase B: bulk per path-block ----------------
            for b in range(NF if "b" in PHASES else 0):
                xs = stack[:, b, 0:n_steps]          # state BEFORE each step
                z1b = bulk.tile([NQ, n_steps], F32, tag="z1b")
                z2b = bulk.tile([NQ, n_steps], F32, tag="z2b")
                nc.sync.dma_start(z1b[:], z1m_v[:, b, 0:n_steps])
                nc.sync.dma_start(z2b[:], z2m_v[:, b, 0:n_steps])

                zs = bulk.tile([NQ, n_steps], F32, tag="zs")
                nc.vector.scalar_tensor_tensor(
                    zs[:], z2b[:], float(C32), z1b[:], Alu.mult, Alu.add)
                vol = bulk.tile([NQ, n_steps], F32, tag="vol")
                nc.scalar.activation(vol[:], xs, Act.Exp, bias=volb[:], scale=0.5)
                sqv = bulk.tile([NQ, n_steps], F32, tag="sqv")
                nc.scalar.activation(
                    sqv[:], vol[:], Act.Square, bias=zerocol[:],
                    scale=float(SQV_SCALE))
                m = bulk.tile([NQ, n_steps], F32, tag="m")
                nc.vector.scalar_tensor_tensor(
                    m[:], vol[:], -1.0, zs[:], Alu.mult, Alu.mult)
                d0 = bulk.tile([NQ, n_steps], F32, tag="d0")
                nc.vector.tensor_scalar(
                    d0[:], sqv[:], -1.0, float(RDT), Alu.mult, Alu.add)
                ls = bout.tile([NQ, n_steps], F32, tag="ls")
                nc.vector.tensor_tensor_scan(
                    ls[:], d0[:], m[:], 0.0, Alu.add, Alu.add)
                spot = bout.tile([NQ, n_steps], F32, tag="spot")
                nc.scalar.activation(spot[:], ls[:], Act.Exp,
                                     bias=zerocol[:], scale=1.0)
                nc.sync.dma_start(sp_v[:, b, 0:n_steps], spot[:])
                nc.sync.dma_start(lv_v[:, b, 0:n_steps], stack[:, b, 1:n_steps + 1])

    _split_waits(nc)
    return nc


def kernel(z1, z2, W1, b1, W2, b2, W3, b3, init_log_v):
    z1 = np.ascontiguousarray(np.asarray(z1, np.float32))
    z2 = np.ascontiguousarray(np.asarray(z2, np.float32))
    gam = _fit_polys(np.asarray(W1), np.asarray(b1), np.asarray(W2),
                     np.asarray(b2), np.asarray(W3), np.asarray(b3),
                     N_STEPS, DEGREE)

    nc = build_program(gam, float(np.asarray(init_log_v)), N_STEPS)

    # sqbias[:, t] = SQ_SCALE * gamma0(t), replicated over partitions
    sqb = np.broadcast_to(
        (SQ_SCALE * gam[:, 0]).astype(np.float32)[None, :], (NQ, N_STEPS))
    sqb = np.ascontiguousarray(sqb)

    in_maps = []
    for c in range(N_CORES):
        sl = slice(c * P_LOCAL, (c + 1) * P_LOCAL)
        z1s = z1[sl]
        in_maps.append({
            "z1m": z1s,
            "z2m": z2[sl],
            "z1t": np.ascontiguousarray(z1s.T),
            "sqbias": sqb,
        })

    res = run_bass_kernel_spmd(nc, in_maps, list(range(N_CORES)))

    lv = np.empty((N_PATHS, N_STEPS), np.float32)
    sp = np.empty((N_PATHS, N_STEPS), np.float32)
    lsq = np.empty((N_PATHS,), np.float32)
    for c in range(N_CORES):
        sl = slice(c * P_LOCAL, (c + 1) * P_LOCAL)
        lv[sl] = res.results[c]["lv"]
        sp[sl] = res.results[c]["sp"]
        lsq[sl] = res.results[c]["lsq"]
    return lv, sp, lsq


# revision 11
# speedup vs baseline: 2.5533x; 2.5533x over previous
"""Trainium2 Bass kernel for the NeuralSDE Q-model scan.

Strategy
--------
Data-parallel over paths: 65536 paths -> 8 cores x 8192 paths, no
cross-core communication. Per core the 8192 paths live as one
(128 partitions x 64 free) tile.

The per-step MLP+tanh (lambda = 3*tanh(MLP(log_v, t))) is replaced by a
per-step degree-D polynomial in the scaled state xi = (log_v + 2.5)/4.5,
fit on host from the (input-provided) MLP weights over the full clipped
range log_v in [-7, 2] (fit max-err ~1e-4 at D=11, ~1e-5 at D=14).

The carried state is xi itself (in [-1, 1]); all affine constants fold
into the polynomial chain / fused-op scalars.

Phase A (sequential 512-step recurrence; one custom-DVE chain per step):
    r   = T_t(xi)                 H3INIT + k x HORNER3   (D = 2+3k)
    v   = alpha*xi + c + r        AFFINE_THEN_ADD
    xi' = clip((s1/h)*z1_t + v)   CLIPADD (bounds -1/1)
    Q  += (c*r)^2                 SQACC
xi' is written into an SBUF-resident history stack (128, 64, 513);
z1 arrives step-major (host-transposed input) so each step reads one
contiguous 32KB row.

Phase B (bulk, per path-block b of 128 paths; steps contiguous):
    x    = h*xi + m               tensor_scalar      -> log_v output rows
    vol  = |c2|*exp(x/2)          ACT Exp (scale h/2, bias m/2+ln|c2|)
    ls   = cumsum_t(R*dt - dt/2*e^x - vol*zs)   LSCAN (fused scan)
    spot = exp(ls)                ACT Exp
where zs = z1 + (c3/c2)*z2 is combined on host (input marshaling).

Outputs stream path-major (contiguous 2KB rows per partition).
"""

import os
import sys

if "/opt/trn_rl_repo" not in sys.path:
    sys.path.insert(0, "/opt/trn_rl_repo")

import numpy as np

import concourse.bass as bass
import concourse.mybir as mybir
import concourse.dve_ops as dve_ops
from concourse import tile
from concourse.bass_utils import run_bass_kernel_spmd
from concourse.dve_spec import (
    Spec, Src0, Src1, C0, C1, C2, AluOp, lower, maxx, minn, sq, scan,
    _has_src1,
)
from concourse.dve_uop import DveOpSpec

Alu = mybir.AluOpType
Act = mybir.ActivationFunctionType
F32 = mybir.dt.float32

# ---- problem constants (hardcoded; must match the reference) ----
N_PATHS = 65536
N_STEPS = 512
DT = 1.0 / 252.0
KAPPA = 2.72
THETA = -3.5
SIGMA_P = (0.1 + 1.6) / 2.0
RHO = -0.85
R = 0.0373
LAMBDA_MAX = 3.0
LOG_V_MIN = -7.0
LOG_V_MAX = 2.0

N_CORES = 8
P_LOCAL = N_PATHS // N_CORES          # 8192
NQ = 128                              # partitions
NF = P_LOCAL // NQ                    # 64 path-blocks (free dim)

# polynomial degree D = 2 + 3k
DEGREE = int(os.environ.get("NSDE_DEGREE", "11"))
assert DEGREE % 3 == 2, "DEGREE must be 2 mod 3 (H3INIT + k*HORNER3)"
XI_M = -2.5                           # x = XI_M + XI_H * xi
XI_H = 4.5

ALPHA = 1.0 - KAPPA * DT
BETA = KAPPA * THETA * DT
S1 = SIGMA_P * np.sqrt(DT)            # noise scale on log_v
CA = LAMBDA_MAX * SIGMA_P * DT        # p~ = -CA * tanh(raw)
SQ_SCALE = 1.0 / (SIGMA_P * np.sqrt(DT))   # (SQ_SCALE*p~)^2 == lambda^2*dt
C2C = RHO * np.sqrt(DT)               # < 0
C3C = np.sqrt(1.0 - RHO * RHO) * np.sqrt(DT)
C32 = C3C / C2C
SQV_SCALE = float(np.sqrt(DT / 2.0) / abs(C2C))  # (SQV_SCALE*vol_s)^2 = dt/2*e^x
RDT = R * DT
# phase-A folded constants (xi-space)
B1_SCALE = ALPHA                                   # v = alpha*xi + B1_BIAS + r/h
B1_BIAS = (ALPHA * XI_M + BETA - XI_M) / XI_H
Z_SCALE = S1 / XI_H
VOL_SCALE = XI_H / 2.0
VOL_BIAS = float(XI_M / 2.0 + np.log(abs(C2C)))

MAX_WAITS = 1

# ---------------------------------------------------------------------------
# custom DVE ops (registered at import; appended to dve_ops.OPS)
# ---------------------------------------------------------------------------

def _register_dve_op(name, spec):
    for op in dve_ops.OPS:
        if op.name == name:
            return op
    row = dve_ops._CUSTOM_DVE_ROW_BASE + len(dve_ops.OPS)
    assert row < 0x20
    dve_ops._SUB_OPCODE_FOR_NAME[name] = row
    shas = {}
    for ver in ("v3", "v4"):
        s = DveOpSpec(name=name, opcode=row, uops=lower(spec, ver=ver),
                      rd1_en=_has_src1(spec))
        shas[ver] = s.sha(ver)
    op = dve_ops.DveOp(name, spec, subdim=False, uops_sha=shas)
    dve_ops.OPS.append(op)
    dve_ops.CUSTOM_DVE_SPECS[name] = spec
    return op


# r = (x*c0 + c1)*x + c2            (degree-2 seed, single tensor stream)
H3INIT = _register_dve_op("NSDE_H3INIT", Spec(
    body=(Src0 * C0 + C1) * Src0 + C2,
    reference=lambda in0, in1, s0, s1, imm2:
        ((in0.astype(np.float32) * np.float32(s0) + np.float32(s1))
         * in0 + np.float32(imm2)).astype(np.float32),
))

# r' = ((r*x + c0)*x + c1)*x + c2   (three Horner steps)
HORNER3 = _register_dve_op("NSDE_HORNER3", Spec(
    body=((Src1 * Src0 + C0) * Src0 + C1) * Src0 + C2,
    reference=lambda in0, in1, s0, s1, imm2:
        (((in1.astype(np.float32) * in0 + np.float32(s0)) * in0
          + np.float32(s1)) * in0 + np.float32(imm2)).astype(np.float32),
))

# out = clip(z*c0 + v, c1, c2)
CLIPADD = _register_dve_op("NSDE_CLIPADD", Spec(
    body=minn(maxx(Src0 * C0 + Src1, C1), C2),
    reference=lambda in0, in1, s0, s1, imm2:
        np.minimum(np.maximum(in0.astype(np.float32) * np.float32(s0) + in1,
                              np.float32(s1)), np.float32(imm2)).astype(np.float32),
))

# Q' = Q + (r*c0)^2
SQACC = _register_dve_op("NSDE_SQACC", Spec(
    body=Src1 + sq(Src0 * C0),
    reference=lambda in0, in1, s0, s1, imm2:
        (in1 + (in0.astype(np.float32) * np.float32(s0)) ** 2).astype(np.float32),
))

# ls = cumsum((c0 - (vol*c1)^2) - vol*zs)
LSCAN = _register_dve_op("NSDE_LSCAN", Spec(
    body=scan(AluOp.ADD, (C0 - sq(Src0 * C1)) - Src0 * Src1),
    reference=lambda in0, in1, s0, s1, imm2:
        np.cumsum((np.float32(s0)
                   - (in0.astype(np.float32) * np.float32(s1)) ** 2)
                  - in0 * in1, axis=-1, dtype=np.float32).astype(np.float32),
))


def _gelu_tanh(x):
    return 0.5 * x * (1.0 + np.tanh(np.sqrt(2 / np.pi) * (x + 0.044715 * x ** 3)))


def _fit_polys(W1, b1, W2, b2, W3, b3, n_steps, degree):
    """Per-step poly coefficients of p~(xi) ~= -CA*tanh(raw(x, t)) in the
    xi power basis; returns (n_steps, degree+1) float64."""
    W1 = W1.astype(np.float64); b1 = b1.astype(np.float64)
    W2 = W2.astype(np.float64); b2 = b2.astype(np.float64)
    W3 = W3.astype(np.float64); b3 = b3.astype(np.float64)
    t_idx = np.arange(n_steps, dtype=np.float64) * DT / (n_steps * DT)

    G = max(4 * (degree + 1), 96)
    k = np.arange(G)
    xg = np.cos(np.pi * (k + 0.5) / G)            # xi nodes in [-1, 1]
    x = XI_M + XI_H * xg

    X = np.stack([np.repeat(x[None, :], n_steps, 0),
                  np.repeat(t_idx[:, None], G, 1)], axis=-1)
    h = _gelu_tanh(X @ W1.T + b1)
    h = _gelu_tanh(h @ W2.T + b2)
    raw = (h @ W3.T + b3)[..., 0]
    f = -CA * np.tanh(raw)

    V = np.polynomial.chebyshev.chebvander(xg, degree)
    coef, *_ = np.linalg.lstsq(V, f.T, rcond=None)
    gam = np.empty((n_steps, degree + 1), np.float64)
    for t in range(n_steps):
        gam[t] = np.polynomial.chebyshev.cheb2poly(coef[:, t])
    return gam


def _split_waits(nc):
    """This walrus build allows only one sync-wait per instruction; push
    extra waits onto preceding same-engine NoOps."""
    for bb in nc.main_func.blocks:
        newlist = []
        for ins in bb.instructions:
            si = ins.sync_info
            if si is not None and si.on_wait and len(si.on_wait) > MAX_WAITS:
                waits = list(si.on_wait)
                extra, keep = waits[:-MAX_WAITS], waits[-MAX_WAITS:]
                for i, w in enumerate(extra):
                    nop = mybir.InstNoOp(
                        name=f"{ins.name}-ws{i}", engine=ins.engine,
                        ins=[], outs=[],
                        sync_info=mybir.SyncInfo(on_wait=[w], on_update=[]))
                    nc.register_instruction(nop, overwrite=True)
                    newlist.append(nop)
                ins.sync_info = mybir.SyncInfo(on_wait=keep,
                                               on_update=list(si.on_update))
            newlist.append(ins)
        bb.instructions[:] = newlist


def chain_consts(gam_t):
    """Map poly coeffs (in xi, p~ units) to the H3INIT/HORNER3 constants
    producing r = T(xi)/XI_H."""
    T = np.asarray(gam_t, np.float64) / XI_H
    D = len(T) - 1
    k = (D - 2) // 3
    init = (float(T[D]), float(T[D - 1]), float(T[D - 2]))
    steps = []
    for j in range(1, k + 1):
        base = 3 * (k - j)
        steps.append((float(T[base + 2]), float(T[base + 1]), float(T[base])))
    return init, steps


def build_program(gam, init_log_v, n_steps):
    """Emit the bass program. gam: (n_steps, D+1) float64, p~ units."""
    nc = bass.Bass()

    zsm = nc.dram_tensor("zsm", [P_LOCAL, N_STEPS], F32, kind="ExternalInput")
    z1t = nc.dram_tensor("z1t", [N_STEPS, P_LOCAL], F32, kind="ExternalInput")
    lv_out = nc.dram_tensor("lv", [P_LOCAL, N_STEPS], F32, kind="ExternalOutput")
    sp_out = nc.dram_tensor("sp", [P_LOCAL, N_STEPS], F32, kind="ExternalOutput")
    q_out = nc.dram_tensor("lsq", [P_LOCAL], F32, kind="ExternalOutput")

    zs_v = zsm.rearrange("(q f) s -> q f s", q=NQ)    # [128, 64, 512]
    lv_v = lv_out.rearrange("(q f) s -> q f s", q=NQ)
    sp_v = sp_out.rearrange("(q f) s -> q f s", q=NQ)
    z1t_v = z1t.rearrange("s (q f) -> s q f", q=NQ)   # [512, 128, 64]
    q_v = q_out.rearrange("(q f) -> q f", q=NQ)       # [128, 64]

    xi0 = (float(init_log_v) - XI_M) / XI_H

    with tile.TileContext(nc) as tc:
        with (
            tc.tile_pool(name="stack", bufs=1) as stack_pool,
            tc.tile_pool(name="state", bufs=1) as state_pool,
            tc.tile_pool(name="work", bufs=2) as work,
            tc.tile_pool(name="zin", bufs=4) as zin,
            tc.tile_pool(name="bulk", bufs=3) as bulk,
            tc.tile_pool(name="bout", bufs=3) as bout,
            tc.tile_pool(name="consts", bufs=1) as consts,
        ):
            stack = stack_pool.tile([NQ, NF, n_steps + 1], F32, tag="stack")
            qacc = state_pool.tile([NQ, NF], F32, tag="qacc")
            zerocol = consts.tile([NQ, 1], F32, tag="zerocol")
            volb = consts.tile([NQ, 1], F32, tag="volb")

            nc.vector.memset(zerocol[:], 0.0)
            nc.vector.memset(volb[:], VOL_BIAS)
            nc.vector.memset(qacc[:], 0.0)
            nc.vector.memset(stack[:, :, 0], xi0)

            # ---------------- Phase A: sequential scan ----------------
            for t in range(n_steps):
                (ia, ib, ic), hsteps = chain_consts(gam[t])
                xt = stack[:, :, t]
                z1tile = zin.tile([NQ, NF], F32, tag="z1tile")
                nc.sync.dma_start(z1tile[:], z1t_v[t])

                r = work.tile([NQ, NF], F32, tag="r")
                nc.vector._custom_dve(H3INIT, out=r[:], in0=xt,
                                      s0=ia, s1=ib, imm2=ic)
                for (d0, d1, d2) in hsteps:
                    nc.vector._custom_dve(HORNER3, out=r[:], in0=xt, in1=r[:],
                                          s0=d0, s1=d1, imm2=d2)
                v = work.tile([NQ, NF], F32, tag="v")
                nc.vector.affine_then_add(v[:], xt, r[:],
                                          float(B1_SCALE), float(B1_BIAS))
                nc.vector._custom_dve(CLIPADD, out=stack[:, :, t + 1],
                                      in0=z1tile[:], in1=v[:],
                                      s0=float(Z_SCALE), s1=-1.0, imm2=1.0)
                nc.vector._custom_dve(SQACC, out=qacc[:], in0=r[:], in1=qacc[:],
                                      s0=float(SQ_SCALE * XI_H))

            nc.sync.dma_start(q_v[:, :], qacc[:])

            # ---------------- Phase B: bulk per path-block ----------------
            for b in range(NF):
                xs_pre = stack[:, b, 0:n_steps]       # xi before each step
                zsb = bulk.tile([NQ, n_steps], F32, tag="zsb")
                nc.sync.dma_start(zsb[:], zs_v[:, b, 0:n_steps])

                xo = bout.tile([NQ, n_steps], F32, tag="xo")
                nc.vector.tensor_scalar(xo[:], stack[:, b, 1:n_steps + 1],
                                        XI_H, XI_M, Alu.mult, Alu.add)
                nc.sync.dma_start(lv_v[:, b, 0:n_steps], xo[:])

                vol = bulk.tile([NQ, n_steps], F32, tag="vol")
                nc.scalar.activation(vol[:], xs_pre, Act.Exp,
                                     bias=volb[:], scale=float(VOL_SCALE))
                ls = bulk.tile([NQ, n_steps], F32, tag="ls")
                nc.vector._custom_dve(LSCAN, out=ls[:], in0=vol[:], in1=zsb[:],
                                      s0=float(RDT), s1=float(SQV_SCALE))
                spot = bout.tile([NQ, n_steps], F32, tag="spot")
                nc.scalar.activation(spot[:], ls[:], Act.Exp,
                                     bias=zerocol[:], scale=1.0)
                nc.sync.dma_start(sp_v[:, b, 0:n_steps], spot[:])

    # Populate .instr bytes for InstCustomDveAnt (raw Bass skips this pass;
    # without it walrus fails with "ISA wrong length").
    mybir.codegen_inst_isa_subclasses(nc)
    _split_waits(nc)
    return nc


def model_numpy(gam, init_log_v, z1s, zss, ns):
    """fp32 numpy model of exactly the device math (for bring-up tests)."""
    D = gam.shape[1] - 1
    xi = np.full((z1s.shape[0],), np.float32((init_log_v - XI_M) / XI_H))
    Q = np.zeros_like(xi)
    ls = np.zeros_like(xi)
    lv = np.zeros((z1s.shape[0], ns), np.float32)
    sp = np.zeros((z1s.shape[0], ns), np.float32)
    for t in range(ns):
        (ia, ib, ic), hsteps = chain_consts(gam[t])
        r = (np.float32(ia) * xi + np.float32(ib)) * xi + np.float32(ic)
        for (d0, d1, d2) in hsteps:
            r = ((r * xi + np.float32(d0)) * xi + np.float32(d1)) * xi + np.float32(d2)
        v = (np.float32(B1_SCALE) * xi + np.float32(B1_BIAS)) + r
        xin = np.minimum(np.maximum(
            np.float32(Z_SCALE) * z1s[:, t] + v, np.float32(-1.0)), np.float32(1.0))
        Q = Q + (np.float32(SQ_SCALE * XI_H) * r) ** 2
        vol = np.exp(np.float32(VOL_SCALE) * xi + np.float32(VOL_BIAS))
        dls = (np.float32(RDT) - (np.float32(SQV_SCALE) * vol) ** 2) - vol * zss[:, t]
        ls = ls + dls
        lv[:, t] = np.float32(XI_H) * xin + np.float32(XI_M)
        sp[:, t] = np.exp(ls)
        xi = xin
    return lv, sp, Q


def kernel(z1, z2, W1, b1, W2, b2, W3, b3, init_log_v):
    z1 = np.ascontiguousarray(np.asarray(z1, np.float32))
    z2 = np.asarray(z2, np.float32)
    zs = np.ascontiguousarray(z1 + np.float32(C32) * z2)
    gam = _fit_polys(np.asarray(W1), np.asarray(b1), np.asarray(W2),
                     np.asarray(b2), np.asarray(W3), np.asarray(b3),
                     N_STEPS, DEGREE)

    nc = build_program(gam, float(np.asarray(init_log_v)), N_STEPS)

    in_maps = []
    for c in range(N_CORES):
        sl = slice(c * P_LOCAL, (c + 1) * P_LOCAL)
        z1s = z1[sl]
        in_maps.append({
            "zsm": zs[sl],
            "z1t": np.ascontiguousarray(z1s.T),
        })

    res = run_bass_kernel_spmd(nc, in_maps, list(range(N_CORES)))

    lv = np.empty((N_PATHS, N_STEPS), np.float32)
    sp = np.empty((N_PATHS, N_STEPS), np.float32)
    lsq = np.empty((N_PATHS,), np.float32)
    for c in range(N_CORES):
        sl = slice(c * P_LOCAL, (c + 1) * P_LOCAL)
        lv[sl] = res.results[c]["lv"]
        sp[sl] = res.results[c]["sp"]
        lsq[sl] = res.results[c]["lsq"]
    return lv, sp, lsq
